# revision 1
# baseline (speedup 1.0000x reference)
"""Trainium2 Bass kernel for rank-1 attention + linear (nn_Attention).

Reference computation (S=256, B=128, D=4096):
    scores   = einsum('sbd,bd->bs', inp, hidden[0])      # dot each enc state with hidden
    attn     = softmax(scores, axis=1)                   # over S
    weighted = einsum('bs,sbd->bd', attn, inp)
    concat   = [weighted, hidden[0]]   # [B, 2D]
    out      = concat @ W.T + b        # [1, B, D]

Distribution over 8 NeuronCores:
  - attention part: data-parallel over B (16 batches per core)
  - linear part: W sharded over output dim (512 rows per core); weighted
    vectors exchanged with an on-chip AllGather.

Per-core dataflow:
  scores  : DVE fused tensor_tensor_reduce (mult + free-dim add) against a
            gpsimd partition-broadcast of the hidden row
  softmax : gpsimd partition_all_reduce (max/add) + ACT exp + DVE reciprocal
  weighted: PE matmuls with a column-masked attn matrix (lhsT [s,16], only
            col b nonzero) accumulating all 16 batches into one PSUM region
  linear  : host-pretransposed W (f-major) + on-chip PE transpose of the
            allgathered weighted matrix; 64 accumulating matmuls
"""

import sys

if "/opt/trn_rl_repo" not in sys.path:
    sys.path.insert(0, "/opt/trn_rl_repo")

import numpy as np


# ----------------------------------------------------------------------------
# Program builder
# ----------------------------------------------------------------------------

def build_program(S=256, B=128, D=4096, n_cores=8, no_collective=False, stage="full",
                  use_f32r=False):
    """Build the SPMD Bass program. Returns finalized nc.

    no_collective=True replaces the AllGather with a local DMA (functionally
    wrong for n_cores>1) so the single-core TimelineSim can model timing.
    stage: "attn" stops after the weighted sums (debug), "attn_ag" adds the
    exchange (debug), "full" is the real kernel.
    """
    import concourse.bacc as bacc
    import concourse.bass_isa as bass_isa
    import concourse.mybir as mybir
    import concourse.tile as tile
    from concourse import library_config

    f32 = mybir.dt.float32
    f32r = mybir.dt.float32r if use_f32r else mybir.dt.float32
    P = 128
    Bc = B // n_cores                 # batches per core
    ST = S // P                       # s-tiles per batch (2)
    F = 2 * D                         # concat feature dim (8192)
    DOUT = D // n_cores               # output-dim shard per core (512)
    NC_D = D // 512                   # 512-wide d-chunks for weighted MMs (8)
    NT_W = D // P                     # 128-wide transpose chunks of weighted (32)
    NT_H = D // P                     # 128-wide chunks of hidden (32)
    NKF = F // P                      # 128-wide k-chunks of the linear (64)

    nc = bacc.Bacc(None, target_bir_lowering=False)

    inp = nc.dram_tensor("inp", [S, Bc, D], f32, kind="ExternalInput")
    hid = nc.dram_tensor("hid", [Bc, D], f32, kind="ExternalInput")
    hidT = nc.dram_tensor("hidT", [D, B], f32, kind="ExternalInput")
    wt = nc.dram_tensor("wt", [F, DOUT], f32, kind="ExternalInput")
    bias = nc.dram_tensor("bias", [1, DOUT], f32, kind="ExternalInput")
    ident = nc.dram_tensor("ident", [P, P], f32, kind="ExternalInput")
    out = nc.dram_tensor("out", [B, DOUT], f32, kind="ExternalOutput")

    cc_in = nc.dram_tensor("cc_in", [Bc, D], f32)
    cc_out = nc.dram_tensor("cc_out", [B, D], f32, addr_space="Shared")

    if stage == "nop":
        with tile.TileContext(nc) as tc:
            with tc.tile_pool(name="sb", bufs=1) as sb:
                t0 = sb.tile([P, 512], f32)
                nc.sync.dma_start(out=t0, in_=inp[0:P, 0, 0:512])
                nc.sync.dma_start(out=out[0:P, 0:512], in_=t0)
                t1 = sb.tile([1, 1], f32)
                nc.sync.dma_start(out=t1, in_=hid[0:1, 0:1])
                t2 = sb.tile([1, 1], f32)
                nc.sync.dma_start(out=t2, in_=hidT[0:1, 0:1])
                t3 = sb.tile([1, 1], f32)
                nc.sync.dma_start(out=t3, in_=wt[0:1, 0:1])
                t4 = sb.tile([1, 1], f32)
                nc.sync.dma_start(out=t4, in_=bias[0:1, 0:1])
                t5 = sb.tile([1, 1], f32)
                nc.sync.dma_start(out=t5, in_=ident[0:1, 0:1])
        nc.finalize()
        return nc

    with tile.TileContext(nc) as tc:
        import contextlib

        with contextlib.ExitStack() as ctx:
            persist = ctx.enter_context(tc.tile_pool(name="persist", bufs=1))

            nc.gpsimd.load_library(library_config.attn)

            ident_sb = persist.tile([P, P], f32)
            nc.sync.dma_start(out=ident_sb, in_=ident[:, :])

            # masked attn weights: [s, t, b, col]; col b of slice (t, b) is
            # batch b's attn column, everything else stays zero
            attn_diag = persist.tile([P, ST, Bc, Bc], f32r)
            nc.vector.memset(attn_diag[:, :, :, :].bitcast(f32), 0.0)

            wsum = persist.tile([Bc, D], f32)

            # linear-stage inputs that stream/land during the batch loop
            wtp = ctx.enter_context(tc.tile_pool(name="wtp", bufs=4))
            hT_sb = persist.tile([P, NT_H, B], f32r)
            nc.sync.dma_start(
                out=hT_sb,
                in_=hidT.rearrange("(c p) b -> p c b", p=P).bitcast(f32r),
            )

            # ---------------- attention (batch loop) ----------------
            with contextlib.ExitStack() as loop_ctx:
                natp = loop_ctx.enter_context(tc.tile_pool(name="nat", bufs=5))
                hbp = loop_ctx.enter_context(tc.tile_pool(name="hb", bufs=2))
                prodp = loop_ctx.enter_context(tc.tile_pool(name="prod", bufs=1))
                smalls = loop_ctx.enter_context(tc.tile_pool(name="smalls", bufs=3))
                waccp = loop_ctx.enter_context(
                    tc.tile_pool(name="wacc", bufs=1, space="PSUM")
                )

                wacc = waccp.tile([Bc, D], f32)

                hrow2 = persist.tile([1, 2, D], f32)

                def emit_hb(b):
                    nc.sync.dma_start(
                        out=hrow2[:, b % 2, :],
                        in_=hid.rearrange("(o b) d -> o b d", o=1)[:, b, :],
                    )
                    hb = hbp.tile([P, D], f32, tag="hb")
                    nc.gpsimd.partition_broadcast(hb, hrow2[:, b % 2, :])
                    return hb

                hbs = {0: emit_hb(0)}

                for b in range(Bc):
                    hb = hbs.pop(b)

                    nats = []
                    sc_b = smalls.tile([P, ST], f32, tag="sc")
                    for t in range(ST):
                        nat = natp.tile([P, D], f32r, tag="nat")
                        nc.sync.dma_start(
                            out=nat, in_=inp[t * P : (t + 1) * P, b, :].bitcast(f32r)
                        )
                        nats.append(nat)
                        prod = prodp.tile([P, D], mybir.dt.bfloat16, tag="prod")
                        nc.vector.scalar_tensor_tensor(
                            out=prod,
                            in0=nat[:, :].bitcast(f32),
                            scalar=1.0,
                            in1=hb,
                            op0=mybir.AluOpType.mult,
                            op1=mybir.AluOpType.mult,
                            accum_out=sc_b[:, t : t + 1],
                        )

                    # prefetch next batch's hidden broadcast so the Pool
                    # FIFO doesn't serialize it behind this batch's PARs
                    if b + 1 < Bc:
                        hbs[b + 1] = emit_hb(b + 1)

                    # softmax over s (partition dim x ST columns)
                    mx2 = smalls.tile([P, ST], f32, tag="mx2")
                    nc.gpsimd.partition_all_reduce(
                        mx2, sc_b, channels=P, reduce_op=bass_isa.ReduceOp.max
                    )
                    negm = smalls.tile([P, 1], f32, tag="negm")
                    nc.vector.tensor_reduce(
                        out=negm, in_=mx2, axis=mybir.AxisListType.X,
                        op=mybir.AluOpType.max, negate=True,
                    )
                    e_b = smalls.tile([P, ST], f32, tag="e_b")
                    s1 = smalls.tile([P, 1], f32, tag="s1")
                    nc.scalar.activation(
                        out=e_b,
                        in_=sc_b,
                        func=mybir.ActivationFunctionType.Exp,
                        bias=negm,
                        scale=1.0,
                        accum_out=s1,
                    )
                    sig = smalls.tile([P, 1], f32, tag="sig")
                    nc.gpsimd.partition_all_reduce(
                        sig, s1, channels=P, reduce_op=bass_isa.ReduceOp.add
                    )
                    r = smalls.tile([P, 1], f32, tag="r")
                    nc.vector.reciprocal(r, sig)
                    attn_b = smalls.tile([P, ST], f32, tag="attn_b")
                    nc.scalar.activation(
                        out=attn_b,
                        in_=e_b,
                        func=mybir.ActivationFunctionType.Copy,
                        bias=0.0,
                        scale=r,
                    )
                    # scatter the two attn columns into their diagonal slots
                    for t in range(ST):
                        nc.scalar.activation(
                            out=attn_diag[:, t, b, b : b + 1],
                            in_=attn_b[:, t : t + 1],
                            func=mybir.ActivationFunctionType.Copy,
                        )

                    # weighted sums: accumulate into wacc rows via masked lhsT
                    for t in range(ST):
                        for c in range(NC_D):
                            nc.tensor.matmul(
                                wacc[:, c * 512 : (c + 1) * 512],
                                attn_diag[:, t, b, :],
                                nats[t][:, c * 512 : (c + 1) * 512],
                                start=(b == 0 and t == 0),
                                stop=(b == Bc - 1 and t == ST - 1),
                            )

                # evacuate weighted PSUM
                nc.scalar.activation(
                    out=wsum, in_=wacc, func=mybir.ActivationFunctionType.Copy
                )

            # early Wt prefetch on the SP queue: SP drains its loop DMAs
            # first, so these transfers land while the last batch computes
            wt_early = {}
            if stage == "full":
                for c in range(NKF // 2, NKF // 2 + 4):
                    wt_sb = wtp.tile([P, DOUT], f32r, tag="wt")
                    nc.sync.dma_start(
                        out=wt_sb, in_=wt[c * P : (c + 1) * P, :].bitcast(f32r)
                    )
                    wt_early[c] = wt_sb

            # ---------------- exchange ----------------
            if stage == "attn":
                nc.sync.dma_start(out=out[:Bc, :DOUT], in_=wsum[:, :DOUT])
            else:
                nc.sync.dma_start(out=cc_in[:, :], in_=wsum)
                if no_collective:
                    for k in range(n_cores):
                        nc.sync.dma_start(
                            out=cc_out[k * Bc : (k + 1) * Bc, :], in_=cc_in[:, :]
                        )
                else:
                    nc.gpsimd.collective_compute(
                        "AllGather",
                        mybir.AluOpType.bypass,
                        replica_groups=[list(range(n_cores))],
                        ins=[cc_in[:, :]],
                        outs=[cc_out[:, :]],
                    )
            if stage == "attn_ag":
                wag_dbg = persist.tile([B, DOUT], f32)
                nc.sync.dma_start(out=wag_dbg, in_=cc_out[:, :DOUT])
                nc.sync.dma_start(out=out[:, :], in_=wag_dbg)

            # ---------------- linear ----------------
            if stage != "full":
                lin_enabled = False
            else:
                lin_enabled = True
            if lin_enabled:
              with contextlib.ExitStack() as lin_ctx:
                tailp = lin_ctx.enter_context(tc.tile_pool(name="tail", bufs=1))
                wTp = lin_ctx.enter_context(tc.tile_pool(name="wTp", bufs=NT_W))
                tpp = lin_ctx.enter_context(
                    tc.tile_pool(name="tp", bufs=4, space="PSUM")
                )
                linp = lin_ctx.enter_context(
                    tc.tile_pool(name="lin", bufs=1, space="PSUM")
                )

                out_ps = linp.tile([B, DOUT], f32)

                # stream all Wt chunks on the ACT DGE queue (SP is busy with
                # the exchange DMAs; separate queue avoids head-of-line block).
                # First few chunks prefetch into the small always-live pool
                # during the batch loop; the bulk goes to a big pool that
                # reuses the loop's SBUF so streaming isn't consumption-gated.
                wtbig = lin_ctx.enter_context(tc.tile_pool(name="wtbig", bufs=24))
                wt_tiles = dict(wt_early)
                for c in list(range(NKF // 2 + 4, NKF)) + list(range(NKF // 2)):
                    wt_sb = wtbig.tile([P, DOUT], f32r, tag="wt")
                    nc.scalar.dma_start(
                        out=wt_sb, in_=wt[c * P : (c + 1) * P, :].bitcast(f32r)
                    )
                    wt_tiles[c] = wt_sb

                # hidden half first: lhsT chunks come straight from hidT input
                for c in range(NKF // 2, NKF):
                    nc.tensor.matmul(
                        out_ps,
                        hT_sb[:, c - NKF // 2, :],
                        wt_tiles.pop(c),
                        start=(c == NKF // 2),
                        stop=False,
                    )

                # weighted half: gather result, transpose on PE, then matmul
                wag = tailp.tile([B, D], f32)
                nc.sync.dma_start(out=wag, in_=cc_out[:, :])

                wTs = []
                for c in range(NT_W):
                    tp_ps = tpp.tile([P, B], f32, tag="tp")
                    nc.tensor.transpose(
                        tp_ps, wag[:, c * P : (c + 1) * P], ident_sb[:B, :B]
                    )
                    wT = wTp.tile([P, B], f32r, tag="wT")
                    nc.vector.tensor_copy(wT, tp_ps)
                    wTs.append(wT)

                for c in range(NT_W):
                    nc.tensor.matmul(
                        out_ps,
                        wTs[c],
                        wt_tiles.pop(c),
                        start=False,
                        stop=(c == NT_W - 1),
                    )

                # bias add + store
                bias_sb = tailp.tile([1, DOUT], f32)
                nc.sync.dma_start(out=bias_sb, in_=bias[:, :])
                bias_bc = tailp.tile([B, DOUT], f32)
                nc.gpsimd.partition_broadcast(bias_bc, bias_sb)
                out_sb = tailp.tile([B, DOUT], f32)
                nc.vector.tensor_add(out_sb, out_ps, bias_bc)
                nc.sync.dma_start(out=out[:, :], in_=out_sb)

    nc.finalize()
    return nc


_CACHE = {}


def _get_program(S, B, D, n_cores):
    key = (S, B, D, n_cores)
    if key not in _CACHE:
        _CACHE[key] = build_program(S, B, D, n_cores)
    return _CACHE[key]


def make_in_maps(inp, hidden, W, b, n_cores=8):
    """Shard host inputs into per-core input maps."""
    S, B, D = inp.shape
    Bc = B // n_cores
    DOUT = W.shape[0] // n_cores
    hidT = np.ascontiguousarray(hidden[0].T)          # [D, B]
    ident = np.eye(128, dtype=np.float32)
    in_maps = []
    for k in range(n_cores):
        in_maps.append(
            {
                "inp": np.ascontiguousarray(inp[:, k * Bc : (k + 1) * Bc, :]),
                "hid": np.ascontiguousarray(hidden[0, k * Bc : (k + 1) * Bc, :]),
                "hidT": hidT,
                "wt": np.ascontiguousarray(W[k * DOUT : (k + 1) * DOUT, :].T),
                "bias": np.ascontiguousarray(
                    b[k * DOUT : (k + 1) * DOUT].reshape(1, DOUT)
                ),
                "ident": ident,
            }
        )
    return in_maps


def kernel(inp, hidden, W, b, trace=False):
    from concourse.bass_utils import run_bass_kernel_spmd

    inp = np.asarray(inp, dtype=np.float32)
    hidden = np.asarray(hidden, dtype=np.float32)
    W = np.asarray(W, dtype=np.float32)
    b = np.asarray(b, dtype=np.float32)

    S, B, D = inp.shape
    n_cores = 8
    nc = _get_program(S, B, D, n_cores)
    in_maps = make_in_maps(inp, hidden, W, b, n_cores)
    res = run_bass_kernel_spmd(nc, in_maps, core_ids=list(range(n_cores)))
    outs = [res.results[k]["out"] for k in range(n_cores)]
    full = np.concatenate(outs, axis=1)  # [B, D]
    if trace:
        return full[None, :, :], res
    return full[None, :, :]



# revision 13
# speedup vs baseline: 1.3920x; 1.3920x over previous
"""Trainium2 Bass kernel for rank-1 attention + linear (nn_Attention).

Reference computation (S=256, B=128, D=4096):
    scores   = einsum('sbd,bd->bs', inp, hidden[0])      # dot each enc state with hidden
    attn     = softmax(scores, axis=1)                   # over S
    weighted = einsum('bs,sbd->bd', attn, inp)
    concat   = [weighted, hidden[0]]   # [B, 2D]
    out      = concat @ W.T + b        # [1, B, D]

Distribution over 8 NeuronCores:
  - attention part: data-parallel over B (16 batches per core)
  - linear part: W sharded over output dim (512 rows per core); weighted
    vectors exchanged with two on-chip AllGathers (batches 0-7 / 8-15) so
    the first exchange overlaps the second half of the batch loop.

All heavy operands are f16 (host-cast): halves HBM traffic and runs the
PE at full rate (f32/f32r matmuls are ~3-4x slower on TRN2).

Per-core dataflow:
  scores  : DVE scalar_tensor_tensor (f16 in, f32 accum) against a gpsimd
            partition-broadcast of the hidden row
  softmax : one gpsimd partition_all_reduce (max) per PAIR of batches +
            ACT exp. NO normalization: the denominator sum(e) is produced
            by an extra ones-column matmul and divided out post-exchange.
  weighted: PE matmuls with column-masked f16 e-vectors (lhsT [s,8], col
            b%8 nonzero). PSUM out rows must start at partition 0/32/64/96
            so the 4096 d-cols are spread over 4 partition-base groups.
  linear  : hidden half from host-pretransposed hidT; weighted half from
            the allgathered, denominator-normalized vectors via PE
            transposes. Output batch rows are in exchange order
            (g,k,j) -> b = k*16+g*8+j; the final store un-permutes.
"""

import sys

if "/opt/trn_rl_repo" not in sys.path:
    sys.path.insert(0, "/opt/trn_rl_repo")

import numpy as np


# ----------------------------------------------------------------------------
# Program builder
# ----------------------------------------------------------------------------

def build_program(S=256, B=128, D=4096, n_cores=8):
    import concourse.bacc as bacc
    import concourse.bass_isa as bass_isa
    import concourse.mybir as mybir
    import concourse.tile as tile
    from concourse import library_config

    f32 = mybir.dt.float32
    f16 = mybir.dt.float16
    P = 128
    Bc = B // n_cores                 # batches per core (16)
    ST = S // P                       # s-tiles per batch (2)
    F = 2 * D                         # concat feature dim (8192)
    DOUT = D // n_cores               # output-dim shard per core (512)
    NKF = F // P                      # 128-wide k-chunks of the linear (64)
    ND = D // P                       # 128-wide d-chunks (32)
    G = Bc // 2                       # batch pairs (8)
    HB = Bc // 2                      # batches per exchange group (8)
    WCC = D + 1                       # exchange payload width (4096 d + den)

    nc = bacc.Bacc(None, target_bir_lowering=False)

    inp = nc.dram_tensor("inp", [Bc, ST, P, D], f16, kind="ExternalInput")
    hid = nc.dram_tensor("hid", [Bc, D], f16, kind="ExternalInput")
    hT = nc.dram_tensor("hT", [P, ND, P], f16, kind="ExternalInput")
    wt = nc.dram_tensor("wt", [P, NKF, DOUT], f16, kind="ExternalInput")
    bias = nc.dram_tensor("bias", [1, DOUT], f32, kind="ExternalInput")
    ident = nc.dram_tensor("ident", [P, P], f32, kind="ExternalInput")
    ones = nc.dram_tensor("ones", [P, 8], f16, kind="ExternalInput")
    out = nc.dram_tensor("out", [B, DOUT], f32, kind="ExternalOutput")

    cc_in = [nc.dram_tensor(f"cc_in{g}", [HB, WCC], f16) for g in range(2)]
    cc_out = [
        nc.dram_tensor(f"cc_out{g}", [n_cores * HB, WCC], f16, addr_space="Shared")
        for g in range(2)
    ]

    inp_r = inp.rearrange("b t p d -> b p t d")

    with tile.TileContext(nc) as tc:
        import contextlib

        with contextlib.ExitStack() as ctx:
            persist = ctx.enter_context(tc.tile_pool(name="persist", bufs=1))

            nc.gpsimd.load_library(library_config.attn)

            # ---- prefetches on the ACT (scalar) HWDGE queue ----
            ident_sb = persist.tile([P, P], f32)
            nc.scalar.dma_start(out=ident_sb, in_=ident[:, :])
            ones_sb = persist.tile([P, 8], f16)
            nc.scalar.dma_start(out=ones_sb, in_=ones[:, :])
            hT_sb = persist.tile([P, ND, P], f16)
            nc.scalar.dma_start(out=hT_sb, in_=hT[:, :, :])
            wt_sb = persist.tile([P, NKF, DOUT], f16)
            for q in range(4):
                nc.scalar.dma_start(
                    out=wt_sb[:, q * 16 : (q + 1) * 16, :],
                    in_=wt[:, q * 16 : (q + 1) * 16, :],
                )
            bias_sb = persist.tile([1, DOUT], f32)
            nc.scalar.dma_start(out=bias_sb, in_=bias[:, :])

            # masked e-vectors: [s, t, col] per 8-batch group; col j of slice
            # (t, j) holds batch (grp*8+j)'s e-values, everything else zero
            diag = persist.tile([P, ST, 8, 8], f16)
            nc.vector.memset(diag[:, :, :, :].bitcast(f32), 0.0)

            # unnormalized weighted sums + denominators, f16, evac dest
            ws = persist.tile([P, 2, 1536], f16)
            den_sb = persist.tile([P, 2], f16)

            hrow2 = persist.tile([1, 2, D], f16)

            # PSUM: banks 0-5. Matmul outputs may only start at partition
            # base 0/32/64, so the 4096 d-cols are spread as 1536/1536/1024
            # over those bases. Per base: group A in cols 0:1536, group B in
            # cols 1536:3072 (base64: A 0:1024, B 1024:2048, denominators in
            # cols 2048 (A) and 2560 (B)).
            waccp = ctx.enter_context(tc.tile_pool(name="wacc", bufs=1, space="PSUM"))
            wacc = waccp.tile([P, 3072], f32)
            # (base, out_col within group, d_lo); n=512 (psum bank limit)
            MM_CHUNKS = [
                (0, 0, 0),
                (0, 512, 512),
                (0, 1024, 1024),
                (32, 0, 1536),
                (32, 512, 2048),
                (32, 1024, 2560),
                (64, 0, 3072),
                (64, 512, 3584),
            ]
            B_OFF = {0: 1536, 32: 1536, 64: 1024}
            DEN_COL = (2048, 2560)
            # evac: (base, cc_col, width)
            EVAC = [(0, 0, 1536), (32, 1536, 1536), (64, 3072, 1024)]

            natp = ctx.enter_context(tc.tile_pool(name="nat", bufs=3))
            hbp = ctx.enter_context(tc.tile_pool(name="hb", bufs=2))
            prodp = ctx.enter_context(tc.tile_pool(name="prod", bufs=1))
            smalls = ctx.enter_context(tc.tile_pool(name="smalls", bufs=2))

            def emit_hb(b):
                nc.sync.dma_start(out=hrow2[:, b % 2, :], in_=hid[b : b + 1, :])
                hb = hbp.tile([P, D], f16, tag="hb")
                nc.gpsimd.partition_broadcast(hb, hrow2[:, b % 2, :])
                return hb

            def evac_group(g):
                for base, cc_col, width in EVAC:
                    src0 = B_OFF[base] if g == 1 else 0
                    nc.scalar.activation(
                        out=ws[base : base + 8, g, 0:width],
                        in_=wacc[base : base + 8, src0 : src0 + width],
                        func=mybir.ActivationFunctionType.Copy,
                    )
                nc.scalar.activation(
                    out=den_sb[64:72, g : g + 1],
                    in_=wacc[64:72, DEN_COL[g] : DEN_COL[g] + 1],
                    func=mybir.ActivationFunctionType.Copy,
                )
                for base, cc_col, width in EVAC:
                    nc.scalar.dma_start(
                        out=cc_in[g][:, cc_col : cc_col + width],
                        in_=ws[base : base + 8, g, 0:width],
                    )
                nc.scalar.dma_start(
                    out=cc_in[g][:, D : D + 1], in_=den_sb[64:72, g : g + 1]
                )

            def emit_allgather(g):
                nc.gpsimd.collective_compute(
                    "AllGather",
                    mybir.AluOpType.bypass,
                    replica_groups=[list(range(n_cores))],
                    ins=[cc_in[g][:, :]],
                    outs=[cc_out[g][:, :]],
                )

            # ---------------- attention (batch loop) ----------------
            hbs = {0: emit_hb(0)}
            nats = {}
            e_tiles = {}
            sc2 = None

            for b in range(Bc):
                grp, j = divmod(b, 8)

                nat = natp.tile([P, ST, D], f16, tag="nat")
                nc.sync.dma_start(out=nat, in_=inp_r[b])
                nats[b] = nat

                hb = hbs[b]
                if b + 1 < Bc:
                    hbs[b + 1] = emit_hb(b + 1)

                if b % 2 == 0:
                    sc2 = smalls.tile([P, 4], f32, tag="sc")
                for t in range(ST):
                    prod = prodp.tile([P, D], f16, tag="prod")
                    nc.vector.scalar_tensor_tensor(
                        out=prod,
                        in0=nat[:, t, :],
                        scalar=1.0,
                        in1=hb,
                        op0=mybir.AluOpType.mult,
                        op1=mybir.AluOpType.mult,
                        accum_out=sc2[:, (b % 2) * 2 + t : (b % 2) * 2 + t + 1],
                    )

                if b % 2 == 1:
                    # one partition all-reduce (max) for the pair
                    mx4 = smalls.tile([P, 4], f32, tag="mx")
                    nc.gpsimd.partition_all_reduce(
                        mx4, sc2, channels=P, reduce_op=bass_isa.ReduceOp.max
                    )
                    for bb in (b - 1, b):
                        o = (bb % 2) * 2
                        negm = smalls.tile([P, 1], f32, tag=f"negm{bb % 2}")
                        nc.vector.tensor_reduce(
                            out=negm, in_=mx4[:, o : o + 2], axis=mybir.AxisListType.X,
                            op=mybir.AluOpType.max, negate=True,
                        )
                        e_b = smalls.tile([P, ST], f16, tag=f"e{bb % 2}")
                        nc.scalar.activation(
                            out=e_b,
                            in_=sc2[:, o : o + 2],
                            func=mybir.ActivationFunctionType.Exp,
                            bias=negm,
                            scale=1.0,
                        )
                        e_tiles[bb] = e_b

                    # weighted-sum matmuls for both batches of the pair
                    for bb in (b - 1, b):
                        gg, jj = divmod(bb, 8)
                        e_b = e_tiles.pop(bb)
                        natb = nats.pop(bb)
                        for t in range(ST):
                            nc.scalar.activation(
                                out=diag[:, t, jj, jj : jj + 1],
                                in_=e_b[:, t : t + 1],
                                func=mybir.ActivationFunctionType.Copy,
                            )
                        for t in range(ST):
                            lhsT = diag[:, t, jj, :]
                            st = jj == 0 and t == 0
                            sp = jj == 7 and t == ST - 1
                            for base, col, d_lo in MM_CHUNKS:
                                co = col + (B_OFF[base] if gg == 1 else 0)
                                nc.tensor.matmul(
                                    wacc[base : base + 8, co : co + 512],
                                    lhsT,
                                    natb[:, t, d_lo : d_lo + 512],
                                    start=st,
                                    stop=sp,
                                )
                            nc.tensor.matmul(
                                wacc[64:72, DEN_COL[gg] : DEN_COL[gg] + 1],
                                lhsT,
                                ones_sb[:, 0:1],
                                start=st,
                                stop=sp,
                            )

                    if b == 7:
                        evac_group(0)
                    if b == 9:
                        emit_allgather(0)
                    if b == Bc - 1:
                        evac_group(1)
                        emit_allgather(1)

            # ---------------- linear tail ----------------
            with contextlib.ExitStack() as lin_ctx:
                tailp = lin_ctx.enter_context(tc.tile_pool(name="tail", bufs=1))
                wnp = lin_ctx.enter_context(tc.tile_pool(name="wn", bufs=2))
                wTp = lin_ctx.enter_context(tc.tile_pool(name="wT", bufs=4))
                tpp = lin_ctx.enter_context(
                    tc.tile_pool(name="tp", bufs=1, space="PSUM")
                )
                linp = lin_ctx.enter_context(
                    tc.tile_pool(name="lin", bufs=1, space="PSUM")
                )

                out_ps = linp.tile([P, DOUT], f32)
                wag = tailp.tile([P, WCC], f16)
                recip = tailp.tile([P, 1], f32)

                # hidden half of the linear: f-chunks 32..63
                for i in range(ND):
                    nc.tensor.matmul(
                        out_ps,
                        hT_sb[:, i, :],
                        wt_sb[:, ND + i, :],
                        start=(i == 0),
                        stop=False,
                        skip_group_check=True,
                    )

                # weighted half, per exchange group (rows g*64:(g+1)*64)
                for g in range(2):
                    r0 = g * 64
                    nc.sync.dma_start(
                        out=wag[r0 : r0 + 64, :], in_=cc_out[g][:, :]
                    )
                    nc.vector.reciprocal(
                        recip[r0 : r0 + 64], wag[r0 : r0 + 64, D : D + 1]
                    )
                    for f in range(4):  # 4 fills of the tp bank, 8 chunks each
                        wn = wnp.tile([P, 1024], f32, tag="wn")
                        for h in range(2):
                            nc.scalar.activation(
                                out=wn[r0 : r0 + 64, h * 512 : (h + 1) * 512],
                                in_=wag[
                                    r0 : r0 + 64,
                                    f * 1024 + h * 512 : f * 1024 + (h + 1) * 512,
                                ],
                                func=mybir.ActivationFunctionType.Copy,
                                scale=recip[r0 : r0 + 64],
                            )
                        tp = tpp.tile([P, DOUT], f32, tag="tp")
                        for q in range(8):
                            nc.tensor.transpose(
                                tp[:, q * 64 : (q + 1) * 64],
                                wn[r0 : r0 + 64, q * P : (q + 1) * P],
                                ident_sb[r0 : r0 + 64, r0 : r0 + 64]
                                if g == 1
                                else ident_sb[0:64, 0:64],
                            )
                        wT = wTp.tile([P, DOUT], f16, tag="wT")
                        nc.vector.tensor_copy(wT, tp)
                        for q in range(8):
                            c = f * 8 + q
                            nc.tensor.matmul(
                                out_ps[r0 : r0 + 64, :],
                                wT[:, q * 64 : (q + 1) * 64],
                                wt_sb[:, c, :],
                                start=False,
                                stop=(c == ND - 1),
                                skip_group_check=True,
                            )

                # bias add + store (un-permute exchange order back to b)
                bias_bc = tailp.tile([P, DOUT], f32)
                nc.gpsimd.partition_broadcast(bias_bc, bias_sb)
                # rows are in exchange order (g,k,j); host un-permutes
                out_sb = tailp.tile([P, DOUT], f32)
                nc.vector.tensor_add(out_sb, out_ps, bias_bc)
                nc.sync.dma_start(out=out[:, :], in_=out_sb)

    nc.finalize()
    return nc


_CACHE = {}


def _get_program(S, B, D, n_cores):
    key = (S, B, D, n_cores)
    if key not in _CACHE:
        _CACHE[key] = build_program(S, B, D, n_cores)
    return _CACHE[key]


def make_in_maps(inp, hidden, W, b, n_cores=8):
    """Shard host inputs into per-core input maps (f16 for heavy operands)."""
    f16 = np.float16
    S, B, D = inp.shape
    Bc = B // n_cores
    DOUT = W.shape[0] // n_cores
    P = 128

    # batch permutation of the exchange order: i=(g,k,j) -> b = k*16+g*8+j
    perm = [k * Bc + g * 8 + j for g in range(2) for k in range(n_cores) for j in range(8)]
    hTg = np.ascontiguousarray(hidden[0].T.astype(f16))          # [D, B]
    hT_pi = hTg[:, perm]                                          # [D, B]
    hT_pack = np.ascontiguousarray(
        hT_pi.reshape(D // P, P, B).transpose(1, 0, 2)
    )                                                             # [P, ND, B]

    ident = np.eye(P, dtype=np.float32)
    ones = np.ones((P, 8), dtype=f16)

    in_maps = []
    for k in range(n_cores):
        inp_k = inp[:, k * Bc : (k + 1) * Bc, :]                  # [S, Bc, D]
        inp_pack = np.ascontiguousarray(
            inp_k.transpose(1, 0, 2).reshape(Bc, 2, P, D).astype(f16)
        )
        wtk = W[k * DOUT : (k + 1) * DOUT, :].T                   # [F, DOUT]
        wt_pack = np.ascontiguousarray(
            wtk.reshape(2 * D // P, P, DOUT).transpose(1, 0, 2).astype(f16)
        )                                                         # [P, NKF, DOUT]
        in_maps.append(
            {
                "inp": inp_pack,
                "hid": np.ascontiguousarray(
                    hidden[0, k * Bc : (k + 1) * Bc, :].astype(f16)
                ),
                "hT": hT_pack,
                "wt": wt_pack,
                "bias": np.ascontiguousarray(
                    b[k * DOUT : (k + 1) * DOUT].reshape(1, DOUT).astype(np.float32)
                ),
                "ident": ident,
                "ones": ones,
            }
        )
    return in_maps


def kernel(inp, hidden, W, b, trace=False):
    from concourse.bass_utils import run_bass_kernel_spmd

    inp = np.asarray(inp, dtype=np.float32)
    hidden = np.asarray(hidden, dtype=np.float32)
    W = np.asarray(W, dtype=np.float32)
    b = np.asarray(b, dtype=np.float32)

    S, B, D = inp.shape
    n_cores = 8
    nc = _get_program(S, B, D, n_cores)
    in_maps = make_in_maps(inp, hidden, W, b, n_cores)
    res = run_bass_kernel_spmd(nc, in_maps, core_ids=list(range(n_cores)))
    # per-core out rows are in exchange order i=(g,k,j) <-> b=k*16+g*8+j
    outs = [
        np.asarray(res.results[k]["out"])
        .reshape(2, n_cores, 8, -1)
        .transpose(1, 0, 2, 3)
        .reshape(B, -1)
        for k in range(n_cores)
    ]
    full = np.concatenate(outs, axis=1)  # [B, D]
    if trace:
        return full[None, :, :], res
    return full[None, :, :]
